# revision 17
# baseline (speedup 1.0000x reference)
"""Trainium2 Bass kernel for nn_DAttention_76579266887926 (deformable sparse attention).

Sharding: data-parallel over B (8 batches -> 8 cores). Each core computes one
batch end-to-end: masked query pooling -> modulated 1x1 conv -> depthwise 3x3
-> LN -> gelu -> offset head -> bilinear grid-sample gather -> K/V projections
-> softmax attention -> output projection.

kernel(**inputs) takes the FULL unsharded inputs and returns
(y, pos, ref) matching reference.reference().
"""

import os
import numpy as np

os.environ.setdefault("MYCRO_LOCAL_CACHE", "1")

import concourse.bacc as bacc
import concourse.bass as bass
import concourse.tile as tile
import concourse.mybir as mybir
from concourse.masks import make_identity

dt = mybir.dt
Alu = mybir.AluOpType
Act = mybir.ActivationFunctionType

# Problem constants (hardcoded per spec)
B = 8
NC = 256
H = W = 32
HW = H * W            # 1024
G = 8                 # n_groups
GC = 32               # group channels
L = 2048
NH = 8                # heads
HC = 32               # head channels
SCALE = HC ** -0.5
ORF = 2.0 / (H - 1.0)   # offset_range * OFFSET_RANGE_FACTOR (same both axes)

F32 = dt.float32
I32 = dt.int32
P = 128


def _v3(ap, u):
    """view (p, N) as (p, N//u, u)"""
    return ap.rearrange("p (a b) -> p a b", b=u)


def build(nc):
    # ---------------- DRAM tensors ----------------
    dr = {}
    dr["q_d"] = nc.dram_tensor("q_b", [L, NC], F32, kind="ExternalInput").ap()
    dr["mask_d"] = nc.dram_tensor("mask_b", [L, 1], F32, kind="ExternalInput").ap()
    dr["x_d"] = nc.dram_tensor("x_b", [NC, HW], F32, kind="ExternalInput").ap()
    dr["Wq_d"] = nc.dram_tensor("Wq", [NC, NC], F32, kind="ExternalInput").ap()
    dr["bq_d"] = nc.dram_tensor("bq", [NC, 1], F32, kind="ExternalInput").ap()
    dr["Wmod_d"] = nc.dram_tensor("Wmod", [NC, NC], F32, kind="ExternalInput").ap()
    dr["dw_w_d"] = nc.dram_tensor("dw_w", [GC, 9], F32, kind="ExternalInput").ap()
    dr["dw_b_d"] = nc.dram_tensor("dw_b", [GC, 1], F32, kind="ExternalInput").ap()
    dr["ln_w_d"] = nc.dram_tensor("ln_w", [GC, 1], F32, kind="ExternalInput").ap()
    dr["ln_b_d"] = nc.dram_tensor("ln_b", [GC, 1], F32, kind="ExternalInput").ap()
    dr["off_w_d"] = nc.dram_tensor("off_w", [2, GC], F32, kind="ExternalInput").ap()
    dr["Wk_d"] = nc.dram_tensor("Wk", [NC, NC], F32, kind="ExternalInput").ap()
    dr["bk_d"] = nc.dram_tensor("bk", [NC, 1], F32, kind="ExternalInput").ap()
    dr["Wv_d"] = nc.dram_tensor("Wv", [NC, NC], F32, kind="ExternalInput").ap()
    dr["bv_d"] = nc.dram_tensor("bv", [NC], F32, kind="ExternalInput").ap()
    dr["Wo_d"] = nc.dram_tensor("Wo", [NC, NC], F32, kind="ExternalInput").ap()
    dr["bo_d"] = nc.dram_tensor("bo", [NC], F32, kind="ExternalInput").ap()

    dr["y_d"] = nc.dram_tensor("y_b", [L, NC], F32, kind="ExternalOutput").ap()
    # pos planes: [0] = y coords, [1] = x coords, each (G, HW)
    dr["pos_d"] = nc.dram_tensor("pos_b", [2, G, HW], F32, kind="ExternalOutput").ap()
    if os.environ.get("ATTN_DEBUG"):
        dr["dbg_xs"] = nc.dram_tensor("dbg_xs", [NC, HW], F32, kind="ExternalOutput").ap()
        dr["dbg_k"] = nc.dram_tensor("dbg_k", [NC, HW], F32, kind="ExternalOutput").ap()
        dr["dbg_vp"] = nc.dram_tensor("dbg_vp", [P, 264], F32, kind="ExternalOutput").ap()
        dr["dbg_on"] = nc.dram_tensor("dbg_on", [NC, L], F32, kind="ExternalOutput").ap()
        dr["dbg_qt"] = nc.dram_tensor("dbg_qt", [NC, L], F32, kind="ExternalOutput").ap()
        dr["dbg_gat"] = nc.dram_tensor("dbg_gat", [P, HW], F32, kind="ExternalOutput").ap()

    with tile.TileContext(nc) as tc:
        _emit(nc, tc, dr)
    return nc


def _emit(nc, tc, dr):
    g = dict(dr)

    with tc.tile_pool(name="perm", bufs=1) as perm, \
         tc.tile_pool(name="mm", bufs=3, space="PSUM") as mm:

        # =========== permanent constants ===========
        ident = perm.tile([P, P], F32, tag="ident", name="ident")
        make_identity(nc, ident[:])

        def load_wT(pool, w_dram, tag, wraw_pool):
            raw = [wraw_pool.tile([P, NC], F32, tag="wraw", name="wraw")
                   for _ in range(2)]
            for o in range(2):
                nc.sync.dma_start(out=raw[o][:], in_=w_dram[o * P:(o + 1) * P, :])
            outs = [pool.tile([P, NC], F32, tag=f"{tag}{i}", name=f"{tag}{i}")
                    for i in range(2)]
            for i in range(2):
                pt = mm.tile([P, NC], F32, tag="mmt", name="wT_ps")
                for o in range(2):
                    nc.tensor.transpose(out=pt[:, o * P:(o + 1) * P],
                                        in_=raw[o][:, i * P:(i + 1) * P],
                                        identity=ident[:])
                nc.vector.tensor_copy(out=outs[i][:], in_=pt[:])
            return outs

        bk_col = [perm.tile([P, 1], F32, tag=f"bk{i}", name=f"bk{i}")
                  for i in range(2)]
        for i in range(2):
            nc.sync.dma_start(out=bk_col[i][:], in_=g["bk_d"][i * P:(i + 1) * P, :])
        bv_bc = perm.tile([P, NC], F32, tag="bv_bc", name="bv_bc")
        nc.gpsimd.dma_start(out=bv_bc[:], in_=bass.AP(
            tensor=g["bv_d"].tensor, offset=0, ap=[[0, P], [1, NC]]))
        bo_bc = perm.tile([P, NC], F32, tag="bo_bc", name="bo_bc")
        nc.gpsimd.dma_start(out=bo_bc[:], in_=bass.AP(
            tensor=g["bo_d"].tensor, offset=0, ap=[[0, P], [1, NC]]))

        qT = [perm.tile([P, L], F32, tag=f"qT{i}", name=f"qT{i}") for i in range(2)]
        xs = [perm.tile([P, HW], F32, tag=f"xs{i}", name=f"xs{i}") for i in range(2)]
        k_sb = [perm.tile([P, HW], F32, tag=f"k{i}", name=f"k{i}") for i in range(2)]
        vp = [perm.tile([P, 264], F32, tag=f"vp{sc}", name=f"vp{sc}")
              for sc in range(8)]
        O_norm = [perm.tile([P, L], F32, tag=f"On{i}", name=f"On{i}")
                  for i in range(2)]

        # =========== stage "pre" pool (spans early + sampling) ====
        with tc.tile_pool(name="pre", bufs=1) as pre:
            posY = pre.tile([8, HW], F32, tag="posY", name="posY")
            posX = pre.tile([8, HW], F32, tag="posX", name="posX")
            x_sb = [pre.tile([P, HW], F32, tag=f"x{i}", name=f"x{i}")
                    for i in range(2)]
            for i in range(2):
                nc.sync.dma_start(out=x_sb[i][:], in_=g["x_d"][i * P:(i + 1) * P, :])

            # ind4[gl, p] = 1.0 if p//32 == gl (for group-replication matmuls)
            ind_i = pre.tile([P, 4], I32, tag="ind_i", name="ind_i")
            nc.gpsimd.iota(ind_i[:], pattern=[[-32, 4]], base=0,
                           channel_multiplier=1)
            ind_f = pre.tile([P, 4], F32, tag="ind_f", name="ind_f")
            nc.vector.tensor_copy(out=ind_f[:], in_=ind_i[:])
            c1 = pre.tile([P, 4], F32, tag="c1", name="c1")
            nc.vector.tensor_scalar(out=c1[:], in0=ind_f[:], scalar1=0.0,
                                    scalar2=None, op0=Alu.is_ge)
            c2 = pre.tile([P, 4], F32, tag="c2", name="c2")
            nc.vector.tensor_scalar(out=c2[:], in0=ind_f[:], scalar1=32.0,
                                    scalar2=None, op0=Alu.is_lt)
            indcol = pre.tile([P, 4], F32, tag="indcol", name="indcol")
            nc.vector.tensor_tensor(out=indcol[:], in0=c1[:], in1=c2[:],
                                    op=Alu.mult)
            i4_ps = mm.tile([4, P], F32, tag="mmt", name="i4_ps")
            nc.tensor.transpose(out=i4_ps[:], in_=indcol[:], identity=ident[:])
            ind4 = pre.tile([4, P], F32, tag="ind4", name="ind4")
            nc.vector.tensor_copy(out=ind4[:], in_=i4_ps[:])

            # =========== stage "early": pooling/modconv/dwconv/LN/offsets ======
            with tc.tile_pool(name="early", bufs=1) as early:
                WkT = load_wT(perm, g["Wk_d"], "WkT", early)
                WvT = load_wT(perm, g["Wv_d"], "WvT", early)
                WoT = load_wT(perm, g["Wo_d"], "WoT", early)
                _emit_early(nc, tc, g, dict(
                    perm=perm, mm=mm, early=early, ident=ident,
                    load_wT=load_wT, qT=qT, posY=posY, posX=posX,
                    x_sb=x_sb, lhsT_sum_col=indcol))

            # =========== sampling: weights/indices, gather, MAC ===========
            with tc.tile_pool(name="samp", bufs=1) as samp:
                _emit_sampling(nc, tc, g, dict(
                    perm=perm, mm=mm, mid=samp, ident=ident, ind4=ind4,
                    posY=posY, posX=posX, xs=xs, x_sb=x_sb))

        # =========== k/v projections ===========
        with tc.tile_pool(name="kv_ps", bufs=2, space="PSUM") as kv_ps:
            for o in range(2):
                for nk in range(2):
                    kp = kv_ps.tile([P, 512], F32, tag="k_ps", name="k_ps")
                    for i in range(2):
                        nc.tensor.matmul(out=kp[:],
                                         lhsT=WkT[i][:, o * P:(o + 1) * P],
                                         rhs=xs[i][:, nk * 512:(nk + 1) * 512],
                                         start=(i == 0), stop=(i == 1))
                    nc.vector.tensor_scalar(
                        out=k_sb[o][:, nk * 512:(nk + 1) * 512], in0=kp[:],
                        scalar1=bk_col[o][:, 0:1], scalar2=None, op0=Alu.add)
            for sc in range(8):
                vps = kv_ps.tile([P, NC], F32, tag="v_ps", name="v_ps")
                for i in range(2):
                    nc.tensor.matmul(out=vps[:], lhsT=xs[i][:, sc * P:(sc + 1) * P],
                                     rhs=WvT[i][:], start=(i == 0), stop=(i == 1))
                vv = vp[sc][:].rearrange("p (h u) -> p h u", u=33)
                nc.vector.tensor_tensor(out=vv[:, :, 0:32], in0=_v3(vps[:], 32),
                                        in1=_v3(bv_bc[:], 32), op=Alu.add)
                nc.vector.memset(vv[:, :, 32:33], 1.0)

        if os.environ.get("ATTN_DEBUG"):
            for i in range(2):
                nc.sync.dma_start(out=g["dbg_xs"][i * P:(i + 1) * P, :], in_=xs[i][:])
                nc.sync.dma_start(out=g["dbg_k"][i * P:(i + 1) * P, :], in_=k_sb[i][:])
                nc.sync.dma_start(out=g["dbg_qt"][i * P:(i + 1) * P, :], in_=qT[i][:])
            nc.sync.dma_start(out=g["dbg_vp"][:], in_=vp[0][:])

        # =========== attention ===========
        with tc.tile_pool(name="apool", bufs=3) as apool, \
             tc.tile_pool(name="aps", bufs=3, space="PSUM") as aps, \
             tc.tile_pool(name="ops", bufs=2, space="PSUM") as ops_pool:
            for h in range(NH):
                kt = k_sb[h // 4]
                prow = slice((h % 4) * 32, (h % 4) * 32 + 32)
                qt = qT[h // 4]
                for lc in range(4):
                    lsl = slice(lc * 512, (lc + 1) * 512)
                    o_ps = ops_pool.tile([33, 512], F32, tag="o_ps", name="o_ps")
                    for sc in range(8):
                        s_ps = aps.tile([P, 512], F32, tag="s_ps", name="s_ps")
                        nc.tensor.matmul(out=s_ps[:],
                                         lhsT=kt[prow, sc * P:(sc + 1) * P],
                                         rhs=qt[prow, lsl], start=True, stop=True,
                                         tile_position=((h % 4) * 32, 0))
                        p_sb = apool.tile([P, 512], F32, tag="p_sb", name="p_sb")
                        nc.scalar.activation(out=p_sb[:], in_=s_ps[:], func=Act.Exp,
                                             bias=0.0, scale=SCALE)
                        nc.tensor.matmul(out=o_ps[:],
                                         lhsT=vp[sc][:, h * 33:h * 33 + 33],
                                         rhs=p_sb[:], start=(sc == 0),
                                         stop=(sc == 7))
                    zr = apool.tile([1, 512], F32, tag="zr", name="zr")
                    nc.vector.reciprocal(out=zr[:], in_=o_ps[32:33, :])
                    zb = apool.tile([32, 512], F32, tag="zb", name="zb")
                    nc.gpsimd.partition_broadcast(zb[:], zr[:])
                    nc.vector.tensor_tensor(out=O_norm[h // 4][prow, lsl],
                                            in0=o_ps[0:32, :], in1=zb[:],
                                            op=Alu.mult)

        if os.environ.get("ATTN_DEBUG"):
            for i in range(2):
                nc.sync.dma_start(out=g["dbg_on"][i * P:(i + 1) * P, :], in_=O_norm[i][:])

        # =========== output projection ===========
        with tc.tile_pool(name="ypool", bufs=3) as ypool:
            for lt in range(16):
                yp = mm.tile([P, NC], F32, tag="mmt", name="y_ps")
                for i in range(2):
                    nc.tensor.matmul(out=yp[:],
                                     lhsT=O_norm[i][:, lt * P:(lt + 1) * P],
                                     rhs=WoT[i][:], start=(i == 0), stop=(i == 1))
                ysb = ypool.tile([P, NC], F32, tag="y_sb", name="y_sb")
                nc.vector.tensor_tensor(out=ysb[:], in0=yp[:], in1=bo_bc[:],
                                        op=Alu.add)
                nc.sync.dma_start(out=g["y_d"][lt * P:(lt + 1) * P, :], in_=ysb[:])


def _emit_early(nc, tc, g, e):
    perm, mm, early, ident = e["perm"], e["mm"], e["early"], e["ident"]
    qT, posY, posX = e["qT"], e["posY"], e["posX"]
    load_wT = e["load_wT"]

    WqT = load_wT(early, g["Wq_d"], "WqT", early)
    WmodT = load_wT(early, g["Wmod_d"], "WmodT", early)
    Wmod2T = [early.tile([P, NC], F32, tag=f"Wmod2T{i}", name=f"Wmod2T{i}")
              for i in range(2)]
    for i in range(2):
        nc.vector.tensor_tensor(out=Wmod2T[i][:], in0=WmodT[i][:],
                                in1=WmodT[i][:], op=Alu.mult)
    bq_col = [early.tile([P, 1], F32, tag=f"bq{i}", name=f"bq{i}")
              for i in range(2)]
    for i in range(2):
        nc.sync.dma_start(out=bq_col[i][:], in_=g["bq_d"][i * P:(i + 1) * P, :])

    # per-partition (p%32) constant columns
    dw_col = early.tile([P, 9], F32, tag="dw_col", name="dw_col")
    dwb_col = early.tile([P, 1], F32, tag="dwb_col", name="dwb_col")
    lnw_col = early.tile([P, 1], F32, tag="lnw_col", name="lnw_col")
    lnb_col = early.tile([P, 1], F32, tag="lnb_col", name="lnb_col")
    for gl in range(4):
        sl = slice(gl * 32, (gl + 1) * 32)
        nc.sync.dma_start(out=dw_col[sl, :], in_=g["dw_w_d"][:])
        nc.sync.dma_start(out=dwb_col[sl, :], in_=g["dw_b_d"][:])
        nc.sync.dma_start(out=lnw_col[sl, :], in_=g["ln_w_d"][:])
        nc.sync.dma_start(out=lnb_col[sl, :], in_=g["ln_b_d"][:])
    offw_col = early.tile([P, 2], F32, tag="offw_col", name="offw_col")
    for gl in range(4):
        nc.sync.dma_start(
            out=offw_col[gl * 32:(gl + 1) * 32, :],
            in_=bass.AP(tensor=g["off_w_d"].tensor, offset=0, ap=[[1, 32], [32, 2]]))

    # LN helpers: ind[p, gl] = (p//32 == gl)
    lhsT_sum = e["lhsT_sum_col"]
    ind_lnw = early.tile([P, 4], F32, tag="ind_lnw", name="ind_lnw")
    nc.vector.tensor_scalar(out=ind_lnw[:], in0=lhsT_sum[:],
                            scalar1=lnw_col[:, 0:1], scalar2=None, op0=Alu.mult)
    rep_ps = mm.tile([4, P], F32, tag="mmt", name="rep_ps")
    nc.tensor.transpose(out=rep_ps[:], in_=ind_lnw[:], identity=ident[:])
    rep_lnw = early.tile([4, P], F32, tag="rep_lnw", name="rep_lnw")
    nc.vector.tensor_copy(out=rep_lnw[:], in_=rep_ps[:])

    # offset-head lhsT, M=40: m = g (y, partitions 0-7), m = 32+g (x, 32-39)
    lhsT_off = [early.tile([P, 40], F32, tag=f"lhsT_off{t}", name=f"lhsT_off{t}")
                for t in range(2)]
    for t in range(2):
        nc.vector.memset(lhsT_off[t][:], 0.0)
        for gl in range(4):
            gg = t * 4 + gl
            psl = slice(gl * 32, (gl + 1) * 32)
            for o in range(2):
                m = o * 32 + gg
                nc.vector.tensor_copy(out=lhsT_off[t][psl, m:m + 1],
                                      in_=offw_col[psl, o:o + 1])

    eps4 = early.tile([4, 1], F32, tag="eps4", name="eps4")
    nc.vector.memset(eps4[:], 1e-5)
    eps1 = early.tile([1, 1], F32, tag="eps1", name="eps1")
    nc.vector.memset(eps1[:], 1e-8)

    x_sb = e["x_sb"]
    refY_n = early.tile([8, HW], F32, tag="refY_n", name="refY_n")
    refX_n = early.tile([8, HW], F32, tag="refX_n", name="refX_n")
    for dst, pat in ((refY_n, [[1, W], [0, W]]), (refX_n, [[0, W], [1, W]])):
        ri = early.tile([8, HW], I32, tag="ref_i", name="ref_i")
        nc.gpsimd.iota(ri[:], pattern=pat, base=0, channel_multiplier=0)
        nc.vector.tensor_copy(out=dst[:], in_=ri[:])
        # r = (0.5+i)/31*2 - 1 = i*(2/31) + (1/31 - 1)
        nc.vector.tensor_scalar(out=dst[:], in0=dst[:],
                                scalar1=2.0 / 31.0, scalar2=1.0 / 31.0 - 1.0,
                                op0=Alu.mult, op1=Alu.add)

    # ---------------- q: pooling + transpose ----------------
    with tc.tile_pool(name="qpool", bufs=4) as qpool, \
         tc.tile_pool(name="pps", bufs=1, space="PSUM") as pps:
        pool_ps = pps.tile([1, NC + 1], F32, tag="pool_ps", name="pool_ps")
        for lt0 in range(0, 16, 4):
            qtiles = []
            for j in range(4):
                lt = lt0 + j
                qt_in = qpool.tile([P, NC + 1], F32, tag="q_in", name="q_in")
                nc.sync.dma_start(out=qt_in[:, 0:NC],
                                  in_=g["q_d"][lt * P:(lt + 1) * P, :])
                nc.vector.memset(qt_in[:, NC:NC + 1], 1.0)
                mt = qpool.tile([P, 1], F32, tag="m_in", name="m_in")
                nc.sync.dma_start(out=mt[:], in_=g["mask_d"][lt * P:(lt + 1) * P, :])
                nc.tensor.matmul(out=pool_ps[:], lhsT=mt[:], rhs=qt_in[:],
                                 start=(lt == 0), stop=(lt == 15))
                qtiles.append(qt_in)
            for chalf in range(2):
                pt = mm.tile([P, 512], F32, tag="mmt", name="qT_ps")
                for j in range(4):
                    nc.tensor.transpose(out=pt[:, j * P:(j + 1) * P],
                                        in_=qtiles[j][:, chalf * P:(chalf + 1) * P],
                                        identity=ident[:])
                nc.vector.tensor_copy(out=qT[chalf][:, lt0 * P:(lt0 + 4) * P],
                                      in_=pt[:])

        # ---------------- conditioning vector ----------------
        pool_sb = early.tile([1, NC + 1], F32, tag="pool_sb", name="pool_sb")
        nc.vector.tensor_copy(out=pool_sb[:], in_=pool_ps[:])

    cnt_r = early.tile([1, 1], F32, tag="cnt_r", name="cnt_r")
    nc.vector.tensor_scalar(out=cnt_r[:], in0=pool_sb[:, NC:NC + 1], scalar1=1e-6,
                            scalar2=None, op0=Alu.add)
    nc.vector.reciprocal(out=cnt_r[:], in_=cnt_r[:])
    cnt_r_col = early.tile([P, 1], F32, tag="cnt_r_col", name="cnt_r_col")
    nc.gpsimd.partition_broadcast(cnt_r_col[:], cnt_r[:])

    pool_col = [early.tile([P, 1], F32, tag=f"pool_col{i}", name=f"pool_col{i}")
                for i in range(2)]
    pc_ps = mm.tile([P, 2], F32, tag="mmt", name="pc_ps")
    for i in range(2):
        nc.tensor.transpose(out=pc_ps[:, i:i + 1],
                            in_=pool_sb[:, i * P:(i + 1) * P],
                            identity=ident[0:1, 0:1])
    for i in range(2):
        nc.vector.tensor_scalar(out=pool_col[i][:], in0=pc_ps[:, i:i + 1],
                                scalar1=cnt_r_col[:, 0:1], scalar2=None,
                                op0=Alu.mult)

    s1_col = [early.tile([P, 1], F32, tag=f"s1_{i}", name=f"s1_{i}")
              for i in range(2)]
    s2_col = [early.tile([P, 1], F32, tag=f"s2_{i}", name=f"s2_{i}")
              for i in range(2)]
    for o in range(2):
        qc_ps = mm.tile([P, 1], F32, tag="mmt", name="qc_ps")
        for i in range(2):
            nc.tensor.matmul(out=qc_ps[:], lhsT=WqT[i][:, o * P:(o + 1) * P],
                             rhs=pool_col[i][:], start=(i == 0), stop=(i == 1))
        nc.vector.tensor_scalar(out=s1_col[o][:], in0=qc_ps[:],
                                scalar1=bq_col[o][:, 0:1], scalar2=1.0,
                                op0=Alu.add, op1=Alu.add)
        nc.vector.tensor_tensor(out=s2_col[o][:], in0=s1_col[o][:],
                                in1=s1_col[o][:], op=Alu.mult)

    dsq_ps = mm.tile([1, NC], F32, tag="mmt", name="dsq_ps")
    for i in range(2):
        nc.tensor.matmul(out=dsq_ps[:], lhsT=s2_col[i][:], rhs=Wmod2T[i][:],
                         start=(i == 0), stop=(i == 1))
    d_row = early.tile([1, NC], F32, tag="d_row", name="d_row")
    nc.scalar.activation(out=d_row[:], in_=dsq_ps[:], func=Act.Sqrt,
                         bias=eps1[:, 0:1], scale=1.0)
    nc.vector.reciprocal(out=d_row[:], in_=d_row[:])
    d_col = [early.tile([P, 1], F32, tag=f"d_col{i}", name=f"d_col{i}")
             for i in range(2)]
    dc_ps = mm.tile([P, 2], F32, tag="mmt", name="dc_ps")
    for i in range(2):
        nc.tensor.transpose(out=dc_ps[:, i:i + 1], in_=d_row[:, i * P:(i + 1) * P],
                            identity=ident[0:1, 0:1])
    for i in range(2):
        nc.vector.tensor_copy(out=d_col[i][:], in_=dc_ps[:, i:i + 1])

    # ---------------- modulated conv ----------------
    x_s = [early.tile([P, HW], F32, tag=f"x_s{i}", name=f"x_s{i}")
           for i in range(2)]
    for i in range(2):
        nc.vector.tensor_scalar(out=x_s[i][:], in0=x_sb[i][:],
                                scalar1=s1_col[i][:, 0:1], scalar2=None,
                                op0=Alu.mult)

    PD = 34
    qt_pad = [early.tile([P, PD * PD], F32, tag=f"qt_pad{i}", name=f"qt_pad{i}")
              for i in range(2)]
    for o in range(2):
        nc.vector.memset(qt_pad[o][:], 0.0)
        for nk in range(2):
            mm_ps = mm.tile([P, 512], F32, tag="mmt", name="mod_ps")
            for i in range(2):
                nc.tensor.matmul(out=mm_ps[:],
                                 lhsT=WmodT[i][:, o * P:(o + 1) * P],
                                 rhs=x_s[i][:, nk * 512:(nk + 1) * 512],
                                 start=(i == 0), stop=(i == 1))
            dst = bass.AP(tensor=qt_pad[o][:].tensor,
                          offset=qt_pad[o][:].offset + (1 + nk * 16) * PD + 1,
                          ap=[qt_pad[o][:].ap[0], [PD, 16], [1, W]])
            nc.vector.tensor_scalar(out=dst, in0=_v3(mm_ps[:], W),
                                    scalar1=d_col[o][:, 0:1], scalar2=None,
                                    op0=Alu.mult)

    # ---------------- depthwise 3x3 ----------------
    def pad_win(t, dy, dx):
        a = t[:]
        return bass.AP(tensor=a.tensor, offset=a.offset + (1 + dy) * PD + (1 + dx),
                       ap=[a.ap[0], [PD, H], [1, W]])

    dwc = [early.tile([P, HW], F32, tag=f"dwc{i}", name=f"dwc{i}")
           for i in range(2)]
    for o in range(2):
        out3 = _v3(dwc[o][:], W)
        nc.vector.tensor_scalar(out=out3, in0=pad_win(qt_pad[o], -1, -1),
                                scalar1=dw_col[:, 0:1], scalar2=dwb_col[:, 0:1],
                                op0=Alu.mult, op1=Alu.add)
        for kk in range(1, 9):
            dy, dx = kk // 3 - 1, kk % 3 - 1
            nc.vector.scalar_tensor_tensor(out=out3, in0=pad_win(qt_pad[o], dy, dx),
                                           scalar=dw_col[:, kk:kk + 1], in1=out3,
                                           op0=Alu.mult, op1=Alu.add)

    # ---------------- layernorm + gelu (per channel half) ----------------
    gel = [early.tile([P, HW], F32, tag=f"gel{i}", name=f"gel{i}")
           for i in range(2)]
    for o in range(2):
        with tc.tile_pool(name=f"stat{o}", bufs=1, space="PSUM") as stat_ps:
            tsq = early.tile([P, HW], F32, tag="tsq", name="tsq")
            nc.vector.tensor_tensor(out=tsq[:], in0=dwc[o][:], in1=dwc[o][:],
                                    op=Alu.mult)
            sum_ps = stat_ps.tile([4, HW], F32, tag="sum_ps", name="sum_ps")
            sq_ps = stat_ps.tile([4, HW], F32, tag="sq_ps", name="sq_ps")
            for nk in range(2):
                sl = slice(nk * 512, (nk + 1) * 512)
                nc.tensor.matmul(out=sum_ps[:, sl], lhsT=lhsT_sum[:],
                                 rhs=dwc[o][:, sl], start=True, stop=True)
                nc.tensor.matmul(out=sq_ps[:, sl], lhsT=lhsT_sum[:],
                                 rhs=tsq[:, sl], start=True, stop=True)
            mean = early.tile([4, HW], F32, tag="ln_mean", name="ln_mean")
            nc.vector.tensor_scalar(out=mean[:], in0=sum_ps[:], scalar1=1.0 / 32.0,
                                    scalar2=None, op0=Alu.mult)
            msq = early.tile([4, HW], F32, tag="ln_msq", name="ln_msq")
            nc.vector.tensor_tensor(out=msq[:], in0=mean[:], in1=mean[:],
                                    op=Alu.mult)
            varp = early.tile([4, HW], F32, tag="ln_varp", name="ln_varp")
            nc.vector.scalar_tensor_tensor(out=varp[:], in0=sq_ps[:],
                                           scalar=1.0 / 32.0, in1=msq[:],
                                           op0=Alu.mult, op1=Alu.subtract)
        nc.scalar.activation(out=varp[:], in_=varp[:], func=Act.Sqrt,
                             bias=eps4[:, 0:1], scale=1.0)
        rstd = varp
        nc.vector.reciprocal(out=rstd[:], in_=rstd[:])
        mrneg = msq
        nc.vector.scalar_tensor_tensor(out=mrneg[:], in0=mean[:], scalar=-1.0,
                                       in1=rstd[:], op0=Alu.mult, op1=Alu.mult)
        for nk in range(2):
            sl = slice(nk * 512, (nk + 1) * 512)
            A_ps = mm.tile([P, 512], F32, tag="mmt", name="A_ps")
            nc.tensor.matmul(out=A_ps[:], lhsT=rep_lnw[:], rhs=rstd[:, sl],
                             start=True, stop=True)
            B_ps = mm.tile([P, 512], F32, tag="mmt", name="B_ps")
            nc.tensor.matmul(out=B_ps[:], lhsT=rep_lnw[:], rhs=mrneg[:, sl],
                             start=True, stop=True)
            tnorm = early.tile([P, 512], F32, tag="tnorm", name="tnorm", bufs=2)
            nc.vector.tensor_tensor(out=tnorm[:], in0=dwc[o][:, sl], in1=A_ps[:],
                                    op=Alu.mult)
            nc.vector.tensor_tensor(out=tnorm[:], in0=tnorm[:], in1=B_ps[:],
                                    op=Alu.add)
            # gelu with per-channel lnb folded into the activation bias
            nc.scalar.activation(out=gel[o][:, sl], in_=tnorm[:], func=Act.Gelu,
                                 bias=lnb_col[:, 0:1], scale=1.0)

    # ---------------- offset head + pos ----------------
    with tc.tile_pool(name="offps", bufs=1, space="PSUM") as offps:
        opt = offps.tile([40, HW], F32, tag="off", name="off")
        for nk in range(2):
            sl = slice(nk * 512, (nk + 1) * 512)
            for o in range(2):
                nc.tensor.matmul(out=opt[:, sl], lhsT=lhsT_off[o][:],
                                 rhs=gel[o][:, sl], start=(o == 0), stop=(o == 1))
        tanh_s = early.tile([40, HW], F32, tag="tanh_s", name="tanh_s")
        nc.scalar.activation(out=tanh_s[:], in_=opt[:], func=Act.Tanh,
                             bias=0.0, scale=1.0)
    nc.vector.scalar_tensor_tensor(out=posY[:], in0=tanh_s[0:8, :], scalar=ORF,
                                   in1=refY_n[:], op0=Alu.mult, op1=Alu.add)
    nc.vector.tensor_scalar(out=posX[:], in0=tanh_s[32:40, :], scalar1=ORF,
                            scalar2=None, op0=Alu.mult)
    nc.vector.tensor_tensor(out=posX[:], in0=posX[:], in1=refX_n[:], op=Alu.add)
    nc.sync.dma_start(out=g["pos_d"][0], in_=posY[:])
    nc.sync.dma_start(out=g["pos_d"][1], in_=posX[:])


def _emit_sampling(nc, tc, g, e):
    perm, mm, mid, ident = e["perm"], e["mm"], e["mid"], e["ident"]
    posY, posX, xs, x_sb = e["posY"], e["posX"], e["xs"], e["x_sb"]
    ind4 = e["ind4"]

    # pixel coords (in place over pos)
    nc.vector.tensor_scalar(out=posY[:], in0=posY[:], scalar1=15.5, scalar2=15.5,
                            op0=Alu.mult, op1=Alu.add)
    nc.vector.tensor_scalar(out=posX[:], in0=posX[:], scalar1=15.5, scalar2=15.5,
                            op0=Alu.mult, op1=Alu.add)

    # ---- sample-partition transposes for the weight pipeline: GT (128, (t,16))
    GT = mid.tile([P, P], F32, tag="GT", name="GT")
    for t in range(8):
        gt_ps = mm.tile([P, 16], F32, tag="mmt", name="gt_ps")
        nc.tensor.transpose(out=gt_ps[:, 0:8], in_=posY[:, t * P:(t + 1) * P],
                            identity=ident[0:8, 0:8])
        nc.tensor.transpose(out=gt_ps[:, 8:16], in_=posX[:, t * P:(t + 1) * P],
                            identity=ident[0:8, 0:8])
        nc.vector.tensor_copy(out=GT[:, t * 16:(t + 1) * 16], in_=gt_ps[:])

    def ftile(tag, shape=(P, P), dtype=F32):
        return mid.tile(list(shape), dtype, tag=tag, name=tag)

    # ---- bilinear weights (with validity), s-part layout (128, (t,g)) ----
    gm = ftile("gm")
    nc.vector.tensor_scalar(out=gm[:], in0=GT[:], scalar1=0.5, scalar2=None,
                            op0=Alu.subtract)
    F0i = ftile("F0i", dtype=I32)
    nc.vector.tensor_copy(out=F0i[:], in_=gm[:])   # HW rounds to nearest
    F0f = ftile("F0f")
    nc.vector.tensor_copy(out=F0f[:], in_=F0i[:])
    frac = ftile("frac")
    nc.vector.tensor_tensor(out=frac[:], in0=GT[:], in1=F0f[:], op=Alu.subtract)
    omf = ftile("omf")
    nc.vector.tensor_scalar(out=omf[:], in0=frac[:], scalar1=-1.0, scalar2=1.0,
                            op0=Alu.mult, op1=Alu.add)
    F0c = ftile("F0c")
    nc.vector.tensor_scalar(out=F0c[:], in0=F0f[:], scalar1=0.0, scalar2=31.0,
                            op0=Alu.max, op1=Alu.min)
    v0 = ftile("v0")
    nc.vector.tensor_tensor(out=v0[:], in0=F0c[:], in1=F0f[:], op=Alu.is_equal)
    F1f = ftile("F1f")
    nc.vector.tensor_scalar(out=F1f[:], in0=F0f[:], scalar1=1.0, scalar2=None,
                            op0=Alu.add)
    F1c = ftile("F1c")
    nc.vector.tensor_scalar(out=F1c[:], in0=F1f[:], scalar1=0.0, scalar2=31.0,
                            op0=Alu.max, op1=Alu.min)
    v1 = ftile("v1")
    nc.vector.tensor_tensor(out=v1[:], in0=F1c[:], in1=F1f[:], op=Alu.is_equal)
    a0 = ftile("a0")
    nc.vector.tensor_tensor(out=a0[:], in0=omf[:], in1=v0[:], op=Alu.mult)
    a1 = ftile("a1")
    nc.vector.tensor_tensor(out=a1[:], in0=frac[:], in1=v1[:], op=Alu.mult)

    def vY(t):
        return _v3(t[:], 16)[:, :, 0:8]

    def vX(t):
        return _v3(t[:], 16)[:, :, 8:16]

    CORNERS = ("00", "01", "10", "11")
    wts = {}
    for cy, ay in (("0", a0), ("1", a1)):
        for cx, ax in (("0", a0), ("1", a1)):
            wt = ftile(f"w{cy}{cx}", shape=(P, 64))
            nc.vector.tensor_tensor(out=_v3(wt[:], 8), in0=vY(ay), in1=vX(ax),
                                    op=Alu.mult)
            wts[cy + cx] = wt

    # ---- weights to group-partition layout is done per-corner in the MAC loop

    # ---- wrapped-index pipeline (16, 512) for ap_gather ----
    # wrapped col layout: (t, q, g); sample s = t*128 + q*16 + u (u = partition)
    with tc.tile_pool(name="wr_ps", bufs=2, space="PSUM") as wr_psp:
        wrY = ftile("wrY", shape=(16, 512))
        wrX = ftile("wrX", shape=(16, 512))
        for src_t, dst in ((posY, wrY), (posX, wrX)):
            wp = wr_psp.tile([16, 512], F32, tag="wr_ps", name="wr_ps")
            for t in range(8):
                for q in range(8):
                    nc.tensor.transpose(
                        out=wp[:, t * 64 + q * 8:t * 64 + q * 8 + 8],
                        in_=src_t[:, t * P + q * 16:t * P + q * 16 + 16],
                        identity=ident[0:8, 0:8])
            nc.vector.tensor_copy(out=dst[:], in_=wp[:])

    def clamp_floor(src):
        gm_w = ftile("gm_w", shape=(16, 512))
        nc.vector.tensor_scalar(out=gm_w[:], in0=src[:], scalar1=0.5,
                                scalar2=None, op0=Alu.subtract)
        i0 = ftile("i0_w", shape=(16, 512), dtype=I32)
        nc.vector.tensor_copy(out=i0[:], in_=gm_w[:])
        f0 = ftile(f"f0_w{id(src) % 97}", shape=(16, 512))
        nc.vector.tensor_copy(out=f0[:], in_=i0[:])
        c0 = ftile(f"c0_w{id(src) % 97}", shape=(16, 512))
        nc.vector.tensor_scalar(out=c0[:], in0=f0[:], scalar1=0.0, scalar2=31.0,
                                op0=Alu.max, op1=Alu.min)
        c1_ = ftile(f"c1_w{id(src) % 97}", shape=(16, 512))
        nc.vector.tensor_scalar(out=c1_[:], in0=f0[:], scalar1=1.0, scalar2=0.0,
                                op0=Alu.add, op1=Alu.max)
        nc.vector.tensor_scalar(out=c1_[:], in0=c1_[:], scalar1=31.0,
                                scalar2=None, op0=Alu.min)
        return c0, c1_

    y0w, y1w = clamp_floor(wrY)
    x0w, x1w = clamp_floor(wrX)

    # idx = y*32 + x per corner -> one int16 tile (16, 2048), col block per corner
    idx_wr = ftile("idx_wr", shape=(16, 4 * 512), dtype=dt.int16)
    idx_f = ftile("idx_f", shape=(16, 512))
    for ci, (yw, xw) in enumerate(((y0w, x0w), (y0w, x1w), (y1w, x0w),
                                   (y1w, x1w))):
        nc.vector.scalar_tensor_tensor(out=idx_f[:], in0=yw[:], scalar=32.0,
                                       in1=xw[:], op0=Alu.mult, op1=Alu.add)
        nc.vector.tensor_copy(out=idx_wr[:, ci * 512:(ci + 1) * 512],
                              in_=idx_f[:])

    # distribute wrapped indices to all 8 16-partition core groups
    # idxs_all[p, (corner, t, q)] = idx_wr[p %% 16, corner*512 + t*64 + q*8 + g(p)]
    idxs_all = ftile("idxs_all", shape=(P, 256), dtype=dt.int16)
    for cg in range(8):
        gg = cg // 2
        src_ap = bass.AP(tensor=idx_wr[:].tensor, offset=idx_wr[:].offset + gg,
                         ap=[idx_wr[:].ap[0], [512, 4], [8, 64]])
        nc.sync.dma_start(out=idxs_all[cg * 16:(cg + 1) * 16, :], in_=src_ap)

    # ---- per corner: weight transpose to group layout, gather, replicate, MAC
    with tc.tile_pool(name="wgp_ps", bufs=2, space="PSUM") as wgp_psp, \
         tc.tile_pool(name="rep_ps", bufs=2, space="PSUM") as rep_psp:
        tmp = mid.tile([P, HW], F32, tag="mac_tmp", name="mac_tmp")
        for ci, key in enumerate(CORNERS):
            w_gp = [mid.tile([4, HW], F32, tag=f"wgp{ch}", name=f"wgp{ch}")
                    for ch in range(2)]
            for ch in range(2):
                for half in range(2):
                    wp = wgp_psp.tile([4, 512], F32, tag="wgp_ps", name="wgp_ps")
                    for tl in range(4):
                        t = half * 4 + tl
                        nc.tensor.transpose(
                            out=wp[:, tl * P:(tl + 1) * P],
                            in_=wts[key][:, t * 8 + ch * 4:t * 8 + ch * 4 + 4],
                            identity=ident[:])
                    nc.vector.tensor_copy(
                        out=w_gp[ch][:, half * 512:(half + 1) * 512], in_=wp[:])
            for ch in range(2):
                gat = mid.tile([P, HW], F32, tag="gat", name="gat", bufs=2)
                nc.gpsimd.ap_gather(
                    out_ap=gat[:], in_ap=x_sb[ch][:],
                    idxs_ap=idxs_all[:, ci * 64:(ci + 1) * 64],
                    channels=P, num_elems=HW, d=1, num_idxs=HW)
                for nk in range(2):
                    sl = slice(nk * 512, (nk + 1) * 512)
                    wrp = rep_psp.tile([P, 512], F32, tag="rep", name="rep")
                    nc.tensor.matmul(out=wrp[:], lhsT=ind4[:],
                                     rhs=w_gp[ch][:, sl], start=True, stop=True)
                    if ci == 0:
                        nc.vector.tensor_tensor(out=xs[ch][:, sl], in0=gat[:, sl],
                                                in1=wrp[:], op=Alu.mult)
                    else:
                        nc.vector.tensor_tensor(out=tmp[:, sl], in0=gat[:, sl],
                                                in1=wrp[:], op=Alu.mult)
                        nc.vector.tensor_tensor(out=xs[ch][:, sl],
                                                in0=xs[ch][:, sl],
                                                in1=tmp[:, sl], op=Alu.add)
        if os.environ.get("ATTN_DEBUG"):
            gat_d = mid.tile([P, HW], F32, tag="gat_d", name="gat_d")
            nc.gpsimd.ap_gather(out_ap=gat_d[:], in_ap=x_sb[0][:],
                                idxs_ap=idxs_all[:, 0:64],
                                channels=P, num_elems=HW, d=1, num_idxs=HW)
            nc.sync.dma_start(out=g["dbg_gat"][:, 0:HW], in_=gat_d[:])


_CACHED = {}


def get_nc():
    if "nc" not in _CACHED:
        nc = bacc.Bacc("TRN2", target_bir_lowering=False, debug=False, num_devices=B)
        build(nc)
        nc.compile()
        _CACHED["nc"] = nc
    return _CACHED["nc"]


def make_in_maps(inputs):
    x = np.ascontiguousarray(inputs["x"], np.float32)
    q = np.ascontiguousarray(inputs["q"], np.float32)
    mask = np.ascontiguousarray(inputs["mask"], np.float32)
    shared = {
        "Wq": np.ascontiguousarray(inputs["Wq"], np.float32),
        "bq": np.ascontiguousarray(inputs["bq"], np.float32).reshape(NC, 1),
        "Wmod": np.ascontiguousarray(inputs["Wmod"], np.float32),
        "dw_w": np.ascontiguousarray(inputs["dw_w"], np.float32).reshape(GC, 9),
        "dw_b": np.ascontiguousarray(inputs["dw_b"], np.float32).reshape(GC, 1),
        "ln_w": np.ascontiguousarray(inputs["ln_w"], np.float32).reshape(GC, 1),
        "ln_b": np.ascontiguousarray(inputs["ln_b"], np.float32).reshape(GC, 1),
        "off_w": np.ascontiguousarray(inputs["off_w"], np.float32),
        "Wk": np.ascontiguousarray(inputs["Wk"], np.float32),
        "bk": np.ascontiguousarray(inputs["bk"], np.float32).reshape(NC, 1),
        "Wv": np.ascontiguousarray(inputs["Wv"], np.float32),
        "bv": np.ascontiguousarray(inputs["bv"], np.float32).reshape(NC),
        "Wo": np.ascontiguousarray(inputs["Wo"], np.float32),
        "bo": np.ascontiguousarray(inputs["bo"], np.float32).reshape(NC),
    }
    in_maps = []
    nref = x.shape[0]
    for b in range(B):
        m = dict(shared)
        m["q_b"] = q[b]
        m["mask_b"] = mask[b]
        m["x_b"] = x[b % nref].reshape(NC, HW)
        in_maps.append(m)
    return in_maps


def make_ref():
    ry = (np.linspace(0.5, H - 0.5, H, dtype=np.float32) / (H - 1.0)) * 2.0 - 1.0
    rx = (np.linspace(0.5, W - 0.5, W, dtype=np.float32) / (W - 1.0)) * 2.0 - 1.0
    ref = np.stack(np.meshgrid(ry, rx, indexing="ij"), -1).astype(np.float32)
    return np.broadcast_to(ref[None, None], (B, G, H, W, 2)).copy()


def assemble_pos(pos_raw):
    # pos_raw: (B, 2, G, HW) planes -> (B, G, H, W, 2)
    return np.moveaxis(pos_raw, 1, -1).reshape(B, G, H, W, 2)


def kernel(**inputs):
    from concourse.bass_utils import run_bass_kernel_spmd

    nc = get_nc()
    in_maps = make_in_maps(inputs)
    br = run_bass_kernel_spmd(nc, in_maps, list(range(B)))
    y = np.stack([br.results[b]["y_b"] for b in range(B)])
    pos = assemble_pos(np.stack([br.results[b]["pos_b"] for b in range(B)]))
    ref = make_ref()
    return y.astype(np.float32), pos.astype(np.float32), ref
